# revision 1
# baseline (speedup 1.0000x reference)
"""DGCNN (2x EdgeConv kNN=5 + MLP head) Trainium2 kernel, data-parallel over 8 NeuronCores.

Contract: kernel(**inputs) takes the FULL inputs of nn_DEC_41180146434796
(pos [32,2048,3] + MLP weights) and returns the FULL [32,2] output.
Each core processes 4 graphs end-to-end (kNN, gathers, max-aggregations local).
"""
import numpy as np

import concourse.bass as bass
import concourse.mybir as mybir
from concourse import bacc, tile
from concourse import bass_utils
from concourse.masks import make_identity

F32 = mybir.dt.float32
F32R = mybir.dt.float32r
U32 = mybir.dt.uint32
I16 = mybir.dt.int16
AF = mybir.ActivationFunctionType
ALU = mybir.AluOpType
AX = mybir.AxisListType

N = 2048          # nodes per graph
NG = 4            # graphs per core
K = 5             # kNN neighbors (incl self)
NT = 16           # node tiles of 128
NCORES = 8

_CACHE = {}


def _sigma_read(ap):
    """View a [C, 2048] natural-ordered tensor so its free stream is sigma-ordered.

    sigma col s = 16*q + b  <->  node i = 128*b + q.  Iteration (q outer, b inner),
    address = 128*b + q.
    """
    return ap.rearrange("c (b q) -> c q b", b=16, q=128)


def build_nc():
    nc = bacc.Bacc(None, target_bir_lowering=False)

    # ---------------- I/O ----------------
    posT_d = nc.dram_tensor("posT", [NG, 3, N], F32, kind="ExternalInput")
    # folded weights (see kernel() for host-side folding)
    w1a_A_d = nc.dram_tensor("w1aA", [3, 64], F32, kind="ExternalInput")
    w1a_B_d = nc.dram_tensor("w1aB", [3, 64], F32, kind="ExternalInput")
    w1b_d = nc.dram_tensor("w1b", [64, 64], F32, kind="ExternalInput")
    w1c_d = nc.dram_tensor("w1c", [64, 64], F32, kind="ExternalInput")
    w2A_d = nc.dram_tensor("w2A", [64, 128], F32, kind="ExternalInput")
    w2B_d = nc.dram_tensor("w2B", [64, 128], F32, kind="ExternalInput")
    wl1_d = nc.dram_tensor("wl1", [64, 1024], F32, kind="ExternalInput")
    wl2_d = nc.dram_tensor("wl2", [128, 1024], F32, kind="ExternalInput")
    wm1_d = nc.dram_tensor("wm1", [128, 8, 512], F32, kind="ExternalInput")
    wm2_d = nc.dram_tensor("wm2", [128, 4, 256], F32, kind="ExternalInput")
    wout_d = nc.dram_tensor("wout", [128, 2, 2], F32, kind="ExternalInput")
    # biases / scales, per-partition layouts
    b1a_d = nc.dram_tensor("b1a", [64, 1], F32, kind="ExternalInput")
    b1b_d = nc.dram_tensor("b1b", [64, 1], F32, kind="ExternalInput")
    b1c_d = nc.dram_tensor("b1c", [64, 1], F32, kind="ExternalInput")
    s1c_d = nc.dram_tensor("s1c", [64, 1], F32, kind="ExternalInput")
    h1c_d = nc.dram_tensor("h1c", [64, 1], F32, kind="ExternalInput")
    b2_d = nc.dram_tensor("b2", [128, 1], F32, kind="ExternalInput")
    bl_d = nc.dram_tensor("bl", [128, 8], F32, kind="ExternalInput")
    bm1_d = nc.dram_tensor("bm1", [128, 4], F32, kind="ExternalInput")
    bm2_d = nc.dram_tensor("bm2", [128, 2], F32, kind="ExternalInput")
    bout_d = nc.dram_tensor("bout", [2, 1], F32, kind="ExternalInput")

    out_d = nc.dram_tensor("out", [2, NG], F32, kind="ExternalOutput")

    with tile.TileContext(nc) as tc:
        with tc.tile_pool(name="wpool", bufs=1) as wp, \
             tc.tile_pool(name="persist", bufs=1) as pp, \
             tc.tile_pool(name="work", bufs=1) as work, \
             tc.tile_pool(name="workB", bufs=3) as workB, \
             tc.tile_pool(name="ps", bufs=2, space="PSUM") as psp:

            # ---------------- weights to SBUF (one-time) ----------------
            def wload(dram, shape, dtype=F32R, name=None):
                t = wp.tile(shape, dtype, name=name or dram.name + "_s")
                if dtype == F32R:
                    fs = 1
                    for d in shape[1:]:
                        fs *= d
                    if len(shape) == 3:
                        tf = t[:].rearrange("p a b -> p (a b)")
                        df = dram[:].rearrange("p a b -> p (a b)")
                    else:
                        tf, df = t[:], dram[:]
                    for o in range(0, fs, 512):
                        w_ = min(512, fs - o)
                        stg = work.tile([shape[0], w_], F32, tag="wstg")
                        nc.sync.dma_start(stg[:], df[:, o:o + w_])
                        nc.scalar.activation(tf[:, o:o + w_], stg[:], AF.Copy)
                else:
                    nc.sync.dma_start(t[:], dram[:])
                return t

            b1a = wload(b1a_d, [64, 1], F32)
            b1b = wload(b1b_d, [64, 1], F32)
            b1c = wload(b1c_d, [64, 1], F32)
            s1c = wload(s1c_d, [64, 1], F32)
            h1c = wload(h1c_d, [64, 1], F32)
            b2 = wload(b2_d, [128, 1], F32)
            bl = wload(bl_d, [128, 8], F32)
            bm1 = wload(bm1_d, [128, 4], F32)
            bm2 = wload(bm2_d, [128, 2], F32)
            bout = wload(bout_d, [2, 1], F32)

            ident = wp.tile([128, 128], F32)
            make_identity(nc, ident[:])
            ones3 = wp.tile([3, 1], F32)
            nc.vector.memset(ones3[:], 1.0)
            ones64 = wp.tile([64, 1], F32)
            nc.vector.memset(ones64[:], 1.0)
            const2 = wp.tile([2, N], F32R)
            nc.vector.memset(const2[:].bitcast(F32), -1.0)
            nc.vector.memset(const2[0:1, :].bitcast(F32), 1.0)
            onesrow = const2[0:1]
            negones = const2[1:2]

            # pooled & relu'd features for the head: [128, mt(8), graph(4)]
            poolr = pp.tile([128, 8, NG], F32R)

            # ============ per-graph pipeline (software-pipelined emission) ============
            ST = {}

            def stageA(g):
                # ---- S0: load pos, round to f32r ----
                posT0 = work.tile([3, N], F32, tag="scrA")
                nc.sync.dma_start(posT0[:], posT_d[g])
                posTr = work.tile([3, N], F32R, tag="posTr")
                nc.scalar.activation(posTr[:], posT0[:], AF.Copy)

                # ---- S1: norms ----
                sq = work.tile([3, N], F32, tag="scrA")
                nc.scalar.activation(sq[:], posTr[:].bitcast(F32), AF.Square)
                x2p = psp.tile([1, N], F32, tag="ps")
                for c in range(4):
                    nc.tensor.matmul(x2p[:, 512 * c:512 * (c + 1)], ones3[:],
                                     sq[:, 512 * c:512 * (c + 1)])
                x2s = work.tile([1, N], F32R, tag="x2s")
                nc.scalar.activation(x2s[:], x2p[:], AF.Copy)
                negx2 = work.tile([1, N], F32R, tag="negx2")
                nc.scalar.activation(negx2[:], x2s[:].bitcast(F32), AF.Copy, scale=-1.0)

                # ---- S2: augmented gram operands [5, N] ----
                rhsA = work.tile([5, N], F32R, tag="rhsA")
                nc.scalar.activation(rhsA[0:3, :], posTr[:].bitcast(F32), AF.Copy)
                nc.sync.dma_start(rhsA[3:4, :], x2s[:])
                nc.sync.dma_start(rhsA[4:5, :], onesrow[:, :])
                lhsA = work.tile([5, N], F32R, tag="lhsA")
                nc.scalar.activation(lhsA[0:3, :], posTr[:].bitcast(F32), AF.Copy, scale=2.0)
                nc.sync.dma_start(lhsA[3:4, :], negones[:, :])
                nc.sync.dma_start(lhsA[4:5, :], negx2[:])

                # ---- S3: gram1 + topk1 ----
                idxall1 = work.tile([128, NT, 8], U32, tag="idxall")
                for t in range(NT):
                    ps = psp.tile([128, N], F32, tag="ps")
                    for c in range(4):
                        nc.tensor.matmul(ps[:, 512 * c:512 * (c + 1)],
                                         lhsA[:, 128 * t:128 * (t + 1)],
                                         rhsA[:, 512 * c:512 * (c + 1)])
                    vals = work.tile([128, 8], F32, tag="vals")
                    nc.vector.max(out=vals[:], in_=ps[:])
                    nc.vector.max_index(out=idxall1[:, t, :], in_max=vals[:], in_values=ps[:])

                # ---- S4: redistribute indices -> wrapped i16 [64, 640] ----
                wrap1 = _make_wrap(nc, tc, work, psp, ident, idxall1,
                                   ngroups=4, tag=f"w1_{g % 2}")
                ST[("wrap1", g)] = wrap1
                ST[("posTr", g)] = posTr

            def stageB(g):
                wrap1 = ST[("wrap1", g)]
                posTr = ST[("posTr", g)]
                # ---- S5: B1 (natural) and A1 (sigma) node features ----
                B1T = work.tile([64, N], F32, tag="BT")
                psb = psp.tile([64, N], F32, tag="ps")
                for c in range(4):
                    nc.tensor.matmul(psb[:, 512 * c:512 * (c + 1)], w1aB[:],
                                     posTr[:, 512 * c:512 * (c + 1)])
                nc.scalar.activation(B1T[:], psb[:], AF.Copy)
                A1s = work.tile([64, N], F32, tag="As")
                psa = psp.tile([64, N], F32, tag="ps")
                sig_pos = _sigma_read(posTr[:])
                for c in range(4):
                    nc.tensor.matmul(psa[:, 512 * c:512 * (c + 1)], w1aA[:],
                                     sig_pos[:, 32 * c:32 * (c + 1), :])
                nc.scalar.activation(A1s[:], psa[:], AF.Copy)

                # ---- S6+S7: conv1 MLP over 5 neighbor slabs ----
                macc = work.tile([64, N], F32, tag="macc")
                for k in range(K):
                    g1 = workB.tile([64, N], F32, tag="gslab")
                    nc.gpsimd.ap_gather(
                        out_ap=g1[:].unsqueeze(-1), in_ap=B1T[:].unsqueeze(-1),
                        idxs_ap=wrap1[:, 128 * k:128 * (k + 1)],
                        channels=64, num_elems=N, d=1, num_idxs=N)
                    nc.vector.tensor_tensor(out=g1[:], in0=g1[:], in1=A1s[:], op=ALU.add)
                    r1a = work.tile([64, N], F32R, tag="r1aslab")
                    nc.scalar.activation(r1a[:], g1[:], AF.Relu, bias=b1a[:])
                    ps1b = psp.tile([64, N], F32, tag="ps")
                    for c in range(4):
                        nc.tensor.matmul(ps1b[:, 512 * c:512 * (c + 1)], w1b[:],
                                         r1a[:, 512 * c:512 * (c + 1)])
                    r1b = work.tile([64, N], F32R, tag="r1bslab")
                    nc.scalar.activation(r1b[:], ps1b[:], AF.Relu, bias=b1b[:])
                    ps1c = psp.tile([64, N], F32, tag="ps")
                    for c in range(4):
                        nc.tensor.matmul(ps1c[:, 512 * c:512 * (c + 1)], w1c[:],
                                         r1b[:, 512 * c:512 * (c + 1)])
                    if k == 0:
                        nc.scalar.activation(macc[:], ps1c[:], AF.Copy)
                    else:
                        nc.vector.tensor_tensor(out=macc[:], in0=macc[:], in1=ps1c[:], op=ALU.max)

                # ---- x1 = bn(relu(macc + b1c)) written natural-order ----
                t1 = work.tile([64, N], F32, tag="scrA")
                nc.scalar.activation(t1[:], macc[:], AF.Relu, bias=b1c[:])
                x1nat = work.tile([64, N], F32R, tag=f"x1nat{g % 2}")
                nc.scalar.activation(
                    _sigma_read(x1nat[:]),
                    t1[:].rearrange("c (q b) -> c q b", q=128, b=16),
                    AF.Identity, bias=h1c[:], scale=s1c[:])

                ST[("x1nat", g)] = x1nat

            def stageC_prep(g):
                x1nat = ST[("x1nat", g)]
                # ---- S8: conv2 norms (natural) ----
                sq1 = work.tile([64, N], F32, tag="scrA")
                nc.scalar.activation(sq1[:], x1nat[:].bitcast(F32), AF.Square)
                x2p2 = psp.tile([1, N], F32, tag="ps")
                for c in range(4):
                    nc.tensor.matmul(x2p2[:, 512 * c:512 * (c + 1)], ones64[:],
                                     sq1[:, 512 * c:512 * (c + 1)])
                x2c = work.tile([1, N], F32R, tag="x2s")
                nc.scalar.activation(x2c[:], x2p2[:], AF.Copy)
                negx2c = work.tile([1, N], F32R, tag="negx2")
                nc.scalar.activation(negx2c[:], x2c[:].bitcast(F32), AF.Copy, scale=-1.0)

                # ---- S9: aug operands [66, N] ----
                rhsA2 = work.tile([66, N], F32R, tag="rhsA")
                nc.scalar.activation(rhsA2[0:64, :], x1nat[:].bitcast(F32), AF.Copy)
                nc.sync.dma_start(rhsA2[64:65, :], x2c[:])
                nc.sync.dma_start(rhsA2[65:66, :], onesrow[:, :])
                lhsA2 = work.tile([66, N], F32R, tag="lhsA")
                nc.scalar.activation(lhsA2[0:64, :], x1nat[:].bitcast(F32), AF.Copy, scale=2.0)
                nc.sync.dma_start(lhsA2[64:65, :], negones[:, :])
                nc.sync.dma_start(lhsA2[65:66, :], negx2c[:])
                ST[("rhsA2", g)] = rhsA2
                ST[("lhsA2", g)] = lhsA2

            def stageC_rest(g):
                rhsA2 = ST[("rhsA2", g)]
                lhsA2 = ST[("lhsA2", g)]
                # ---- S10: gram2 + topk2 ----
                idxall2 = work.tile([128, NT, 8], U32, tag="idxall")
                for t in range(NT):
                    ps = psp.tile([128, N], F32, tag="ps")
                    for c in range(4):
                        nc.tensor.matmul(ps[:, 512 * c:512 * (c + 1)],
                                         lhsA2[:, 128 * t:128 * (t + 1)],
                                         rhsA2[:, 512 * c:512 * (c + 1)])
                    vals2 = work.tile([128, 8], F32, tag="vals")
                    nc.vector.max(out=vals2[:], in_=ps[:])
                    nc.vector.max_index(out=idxall2[:, t, :], in_max=vals2[:], in_values=ps[:])

                # ---- S11: redistribute ----
                wrap2 = _make_wrap(nc, tc, work, psp, ident, idxall2,
                                   ngroups=8, tag=f"w2_{g % 2}")
                ST[("wrap2", g)] = wrap2

            def stageD(g):
                wrap2 = ST[("wrap2", g)]
                x1nat = ST[("x1nat", g)]
                sig_x1 = _sigma_read(x1nat[:])
                # ---- S12: B2 (natural), A2 (sigma) ----
                B2T = work.tile([128, N], F32, tag="B2T")
                psb2 = psp.tile([128, N], F32, tag="ps")
                for c in range(4):
                    nc.tensor.matmul(psb2[:, 512 * c:512 * (c + 1)], w2B[:],
                                     x1nat[:, 512 * c:512 * (c + 1)])
                nc.scalar.activation(B2T[:], psb2[:], AF.Copy)
                A2s = work.tile([128, N], F32, tag="A2s")
                psa2 = psp.tile([128, N], F32, tag="ps")
                sig_x1 = _sigma_read(x1nat[:])
                for c in range(4):
                    nc.tensor.matmul(psa2[:, 512 * c:512 * (c + 1)], w2A[:],
                                     sig_x1[:, 32 * c:32 * (c + 1), :])
                nc.scalar.activation(A2s[:], psa2[:], AF.Copy)

                # ---- S13+S14: gather-max + combine ----
                macc2 = work.tile([128, N], F32, tag="macc")
                for k in range(K):
                    g2 = workB.tile([128, N], F32, tag="gslab")
                    nc.gpsimd.ap_gather(
                        out_ap=g2[:].unsqueeze(-1), in_ap=B2T[:].unsqueeze(-1),
                        idxs_ap=wrap2[:, 128 * k:128 * (k + 1)],
                        channels=128, num_elems=N, d=1, num_idxs=N)
                    if k == 0:
                        nc.scalar.activation(macc2[:], g2[:], AF.Copy)
                    else:
                        nc.vector.tensor_tensor(out=macc2[:], in0=macc2[:], in1=g2[:], op=ALU.max)
                nc.vector.tensor_tensor(out=macc2[:], in0=macc2[:], in1=A2s[:], op=ALU.add)
                x2sg = work.tile([128, N], F32R, tag="x2sg")
                nc.scalar.activation(x2sg[:], macc2[:], AF.Relu, bias=b2[:])

                ST[("x2sg", g)] = x2sg

            def stageE(g):
                x2sg = ST[("x2sg", g)]
                x1nat = ST[("x1nat", g)]
                sig_x1 = _sigma_read(x1nat[:])
                # ---- S15: linear-l + global max pool ----
                for mt in range(8):
                    psl = psp.tile([128, N], F32, tag="ps")
                    for c in range(4):
                        nc.tensor.matmul(psl[:, 512 * c:512 * (c + 1)],
                                         wl1[:, 128 * mt:128 * (mt + 1)],
                                         sig_x1[:, 32 * c:32 * (c + 1), :],
                                         start=True, stop=False)
                    for c in range(4):
                        nc.tensor.matmul(psl[:, 512 * c:512 * (c + 1)],
                                         wl2[:, 128 * mt:128 * (mt + 1)],
                                         x2sg[:, 512 * c:512 * (c + 1)],
                                         start=False, stop=True)
                    pr = work.tile([128, 1], F32, tag=f"poolred{mt % 2}")
                    nc.vector.tensor_reduce(pr[:], psl[:], axis=AX.X, op=ALU.max)
                    nc.scalar.activation(poolr[:, mt, g:g + 1], pr[:],
                                         AF.Relu, bias=bl[:, mt:mt + 1])

            def stageC(g):
                stageC_prep(g)
                stageC_rest(g)

            stageA(0)
            # f32r weight staging overlaps topk1(0) on the Act stream
            w1aA = wload(w1a_A_d, [3, 64])
            w1aB = wload(w1a_B_d, [3, 64])
            w1b = wload(w1b_d, [64, 64])
            w1c = wload(w1c_d, [64, 64])
            w2A = wload(w2A_d, [64, 128])
            w2B = wload(w2B_d, [64, 128])
            wl1 = wload(wl1_d, [64, 1024])
            wl2 = wload(wl2_d, [128, 1024])
            wm1 = wload(wm1_d, [128, 8, 512])
            wm2 = wload(wm2_d, [128, 4, 256])
            wout = wload(wout_d, [128, 2, 2])
            stageB(0)
            stageC(0)
            for g in range(NG):
                if g + 1 < NG:
                    stageA(g + 1)
                stageD(g)
                if g + 1 < NG:
                    stageB(g + 1)
                stageE(g)
                if g + 1 < NG:
                    stageC(g + 1)

            # ============ head MLP (all graphs) ============
            rm1 = pp.tile([128, 4, NG], F32R)
            for mt in range(4):
                ph = psp.tile([128, NG], F32, tag="ps")
                for kk in range(8):
                    nc.tensor.matmul(ph[:], wm1[:, kk, 128 * mt:128 * (mt + 1)],
                                     poolr[:, kk, :], start=(kk == 0), stop=(kk == 7))
                nc.scalar.activation(rm1[:, mt, :], ph[:], AF.Relu,
                                     bias=bm1[:, mt:mt + 1])
            rm2 = pp.tile([128, 2, NG], F32R)
            for mt in range(2):
                ph = psp.tile([128, NG], F32, tag="ps")
                for kk in range(4):
                    nc.tensor.matmul(ph[:], wm2[:, kk, 128 * mt:128 * (mt + 1)],
                                     rm1[:, kk, :], start=(kk == 0), stop=(kk == 3))
                nc.scalar.activation(rm2[:, mt, :], ph[:], AF.Relu,
                                     bias=bm2[:, mt:mt + 1])
            pho = psp.tile([2, NG], F32, tag="ps")
            for kk in range(2):
                nc.tensor.matmul(pho[:], wout[:, kk, :], rm2[:, kk, :],
                                 start=(kk == 0), stop=(kk == 1))
            outs = pp.tile([2, NG], F32)
            nc.vector.tensor_scalar_add(outs[:], pho[:], bout[:])
            nc.sync.dma_start(out_d[:], outs[:])

    nc.compile()
    return nc


def _make_wrap(nc, tc, work, psp, ident, idxall, ngroups, tag):
    """[128, 16, 8] u32 find_index8 results -> wrapped i16 [16*ngroups, 640] for ap_gather.

    Edge order m = 2048*k + 16*q + b: node i = 128*b + q, slot k.
    """
    F32_ = mybir.dt.float32
    I16_ = mybir.dt.int16
    idxf = work.tile([128, 5, 16], F32_, tag=tag + "idxf")
    nc.vector.tensor_copy(idxf[:], idxall[:, :, 0:5].transpose([0, 2, 1]))
    tp = psp.tile([80, 128], F32_, tag="ps")
    nc.tensor.transpose(tp[:], idxf[:].rearrange("p a b -> p (a b)"), ident[:])
    idxt16 = work.tile([80, 128], I16_, tag=tag + "idxt16")
    nc.vector.tensor_copy(idxt16[:], tp[:])
    wrap = work.tile([16 * ngroups, 640], I16_, tag=tag + "wrap")
    for gg in range(ngroups):
        for k in range(5):
            nc.sync.dma_start(wrap[16 * gg:16 * (gg + 1), 128 * k:128 * (k + 1)],
                              idxt16[16 * k:16 * k + 16, :])
    return wrap


def _fold_weights(inp):
    """Host-side BN folding / edge-weight splitting. Layout-only + tiny weight algebra."""
    f = {k: np.asarray(v, dtype=np.float64) for k, v in inp.items()}
    w = {}
    # conv1 layer a: e @ W1a = x_i @ (Wtop - Wbot) + x_j @ Wbot
    w["w1aA"] = (f["w1a"][:3] - f["w1a"][3:])
    w["w1aB"] = f["w1a"][3:]
    w["b1a"] = f["b1a"]
    # fold (s1a, h1a) into layer b; (s1b, h1b) into layer c
    w["w1b"] = f["s1a"][:, None] * f["w1b"]
    w["b1b"] = f["h1a"] @ f["w1b"] + f["b1b"]
    w["w1c"] = f["s1b"][:, None] * f["w1c"]
    w["b1c"] = f["h1b"] @ f["w1c"] + f["b1c"]
    w["s1c"], w["h1c"] = f["s1c"], f["h1c"]
    # conv2
    w["w2A"] = f["w2"][:64] - f["w2"][64:]
    w["w2B"] = f["w2"][64:]
    w["b2"] = f["b2"]
    # linear l: x1-part plain; x2-part folded with (s2, h2)
    wl1 = f["wl"][:64]
    wl2 = f["s2"][:, None] * f["wl"][64:]
    blf = f["bl"] + f["h2"] @ f["wl"][64:]
    w["wl1"], w["wl2"], w["bl"] = wl1, wl2, blf
    # head: fold (sl, hl) into m1; (sm1, hm1) into m2; (sm2, hm2) into out
    w["wm1"] = f["sl"][:, None] * f["wm1"]
    w["bm1"] = f["hl"] @ f["wm1"] + f["bm1"]
    w["wm2"] = f["sm1"][:, None] * f["wm2"]
    w["bm2"] = f["hm1"] @ f["wm2"] + f["bm2"]
    w["wout"] = f["sm2"][:, None] * f["wout"]
    w["bout"] = f["hm2"] @ f["wout"] + f["bout"]
    return {k: v.astype(np.float32) for k, v in w.items()}


def _weight_maps(w):
    m = {}
    m["w1aA"] = w["w1aA"]
    m["w1aB"] = w["w1aB"]
    m["w1b"] = w["w1b"]
    m["w1c"] = w["w1c"]
    m["w2A"] = w["w2A"]
    m["w2B"] = w["w2B"]
    m["wl1"] = w["wl1"]
    m["wl2"] = w["wl2"]
    m["wm1"] = np.ascontiguousarray(w["wm1"].reshape(8, 128, 512).transpose(1, 0, 2))
    m["wm2"] = np.ascontiguousarray(w["wm2"].reshape(4, 128, 256).transpose(1, 0, 2))
    m["wout"] = np.ascontiguousarray(w["wout"].reshape(2, 128, 2).transpose(1, 0, 2))
    m["b1a"] = w["b1a"].reshape(64, 1)
    m["b1b"] = w["b1b"].reshape(64, 1)
    m["b1c"] = w["b1c"].reshape(64, 1)
    m["s1c"] = w["s1c"].reshape(64, 1)
    m["h1c"] = w["h1c"].reshape(64, 1)
    m["b2"] = w["b2"].reshape(128, 1)
    m["bl"] = np.ascontiguousarray(w["bl"].reshape(8, 128).T)
    m["bm1"] = np.ascontiguousarray(w["bm1"].reshape(4, 128).T)
    m["bm2"] = np.ascontiguousarray(w["bm2"].reshape(2, 128).T)
    m["bout"] = w["bout"].reshape(2, 1)
    return {k: np.ascontiguousarray(v, dtype=np.float32) for k, v in m.items()}


def kernel(**inputs):
    if "nc" not in _CACHE:
        _CACHE["nc"] = build_nc()
    nc = _CACHE["nc"]

    w = _fold_weights(inputs)
    wm = _weight_maps(w)
    pos = np.asarray(inputs["pos"], dtype=np.float32)  # [32, 2048, 3]
    B = pos.shape[0]

    in_maps = []
    for c in range(NCORES):
        m = dict(wm)
        m["posT"] = np.ascontiguousarray(pos[NG * c:NG * (c + 1)].transpose(0, 2, 1))
        in_maps.append(m)

    res = bass_utils.run_bass_kernel_spmd(nc, in_maps, core_ids=list(range(NCORES)))
    out = np.zeros((B, 2), dtype=np.float32)
    for c in range(NCORES):
        out[NG * c:NG * (c + 1)] = res.results[c]["out"].T
    return out



# revision 7
# speedup vs baseline: 14.7640x; 14.7640x over previous
"""DGCNN (2x EdgeConv kNN=5 + MLP head) Trainium2 kernel, data-parallel over 8 NeuronCores.

Contract: kernel(**inputs) takes the FULL inputs of nn_DEC_41180146434796
(pos [32,2048,3] + MLP weights) and returns the FULL [32,2] output.
Each core processes 4 graphs end-to-end (kNN, gathers, max-aggregations local).

Per-call cost optimizations vs the unrolled baseline:
- weights ship as ONE fp16 blob sharded 1/8 per core and are AllGather'd
  on-device (axon input transfer is the dominant cost at ~76 MB/s);
- the per-graph pipeline is emitted once inside hardware For_i loops
  (per-call cost also scales with NEFF instruction count).
"""
import numpy as np

import concourse.bass as bass
import concourse.mybir as mybir
from concourse import bacc, tile
from concourse import bass_utils
from concourse.bass import ds
from concourse.masks import make_identity

F32 = mybir.dt.float32
F32R = mybir.dt.float32r
F16 = mybir.dt.float16
U32 = mybir.dt.uint32
I16 = mybir.dt.int16
AF = mybir.ActivationFunctionType
ALU = mybir.AluOpType
AX = mybir.AxisListType

N = 2048          # nodes per graph
NG = 4            # graphs per core
K = 5             # kNN neighbors (incl self)
NT = 16           # node tiles of 128
NCORES = 8

# packed fp16 weight wall column offsets ([128, XT])
C_W1AA, C_W1AB = 0, 64            # [3, 64] each
C_W1B, C_W1C = 128, 192           # [64, 64] each
C_W2A, C_W2B = 256, 384           # [64, 128] each
C_WL1 = 512                       # [64, 1024]
C_WL2 = 1536                      # [128, 1024]
C_WM1 = 2560                      # [128, 8, 512]
C_WM2 = 6656                      # [128, 4, 256]
C_WOUT = 7680                     # [128, 2, 2]
C_BIAS = 7684                     # 21 cols of biases (see _pack_wall)
XT = 7712

_CACHE = {}


def build_nc():
    nc = bacc.Bacc(None, target_bir_lowering=False, num_devices=NCORES)

    posT_d = nc.dram_tensor("posT", [NG, 3, N], F32, kind="ExternalInput")
    wsh_d = nc.dram_tensor("wsh", [16, XT], F16, kind="ExternalInput")
    out_d = nc.dram_tensor("out", [2, NG], F32, kind="ExternalOutput")

    with tile.TileContext(nc) as tc:
        with tc.tile_pool(name="dramp", bufs=1, space="DRAM") as dramp, \
             tc.tile_pool(name="wp", bufs=1) as wp, \
             tc.tile_pool(name="work", bufs=1) as work, \
             tc.tile_pool(name="scr2", bufs=2) as scr2, \
             tc.tile_pool(name="ps", bufs=2, space="PSUM") as psp:

            # ---- weights: shard -> AllGather -> SBUF, fp16 -> f32r/f32 ----
            inb = dramp.tile([16, XT], F16, name="inb")
            outb = dramp.tile([128, XT], F16, name="outb")
            nc.gpsimd.dma_start(inb[:], wsh_d[:])
            nc.gpsimd.collective_compute(
                "AllGather", ALU.bypass,
                replica_groups=[list(range(NCORES))],
                ins=[inb.opt()], outs=[outb.opt()])
            wall = wp.tile([128, XT], F32R, name="wall")
            biasw = wp.tile([128, 21], F32, name="biasw")
            with tc.tile_pool(name="wstg", bufs=1) as wstgp:
                wall_h = wstgp.tile([128, XT], F16, name="wall_h")
                nc.sync.dma_start(wall_h[:], outb[:])
                for o in range(0, XT, 1928):
                    nc.scalar.activation(wall[:, o:o + 1928], wall_h[:, o:o + 1928],
                                         AF.Copy)
                nc.scalar.activation(biasw[:], wall_h[:, C_BIAS:C_BIAS + 21], AF.Copy)

            w1aA = wall[0:3, C_W1AA:C_W1AA + 64]
            w1aB = wall[0:3, C_W1AB:C_W1AB + 64]
            w1b = wall[0:64, C_W1B:C_W1B + 64]
            w1c = wall[0:64, C_W1C:C_W1C + 64]
            w2A = wall[0:64, C_W2A:C_W2A + 128]
            w2B = wall[0:64, C_W2B:C_W2B + 128]
            wl1 = wall[0:64, C_WL1:C_WL1 + 1024]
            wl2 = wall[0:128, C_WL2:C_WL2 + 1024]
            wm1 = wall[:, C_WM1:C_WM1 + 4096].rearrange("p (a b) -> p a b", a=8, b=512)
            wm2 = wall[:, C_WM2:C_WM2 + 1024].rearrange("p (a b) -> p a b", a=4, b=256)
            wout = wall[:, C_WOUT:C_WOUT + 4].rearrange("p (a b) -> p a b", a=2, b=2)
            b1a = biasw[0:64, 0:1]
            b1b = biasw[0:64, 1:2]
            b1c = biasw[0:64, 2:3]
            s1c = biasw[0:64, 3:4]
            h1c = biasw[0:64, 4:5]
            b2 = biasw[0:128, 5:6]
            # bl: cols 6..14, bm1: 14..18, bm2: 18..20, bout: 20
            bout = biasw[0:2, 20:21]

            # ---- constants ----
            ident = wp.tile([128, 128], F32, name="ident")
            make_identity(nc, ident[:])
            ones3 = wp.tile([3, 1], F32, name="ones3")
            nc.vector.memset(ones3[:], 1.0)
            ones64 = wp.tile([64, 1], F32, name="ones64")
            nc.vector.memset(ones64[:], 1.0)
            const2 = wp.tile([2, N], F32R, name="const2")
            nc.vector.memset(const2[:].bitcast(F32), -1.0)
            nc.vector.memset(const2[0:1, :].bitcast(F32), 1.0)
            onesrow = const2[0:1]
            negones = const2[1:2]

            # all graphs' positions resident in SBUF: [3, NG*N]
            posTall = wp.tile([3, NG * N], F32, name="posTall")
            nc.sync.dma_start(
                posTall[:].rearrange("c (g n) -> c g n", g=NG, n=N),
                posT_d[:].rearrange("g c n -> c g n"))

            # pooled features accumulated per graph: [128, 8, NG]
            poolall = wp.tile([128, 8, NG], F32R, name="poolall")

            # ---------------- per-graph pipeline (hardware loop) ----------------
            def emit_knn(featr, rows, onesv, ngroups, nm):
                """kNN via augmented gram matmul + top-8; returns wrapped i16 idx."""
                sqb = scr2.tile([64, N], F32, tag="scr", name=nm + "sq")
                sq = sqb[0:rows, :]
                nc.scalar.activation(sq, featr[:].bitcast(F32), AF.Square)
                x2p = psp.tile([1, N], F32, tag="ps", name=nm + "x2p")
                for c in range(4):
                    nc.tensor.matmul(x2p[:, 512 * c:512 * (c + 1)], onesv[:],
                                     sq[:, 512 * c:512 * (c + 1)])
                x2sb = scr2.tile([64, N], F32, tag="scr", name=nm + "x2s")
                x2s = x2sb[0:1, :].bitcast(F32R)
                nc.scalar.activation(x2s, x2p[:], AF.Copy)
                negx2b = scr2.tile([64, N], F32, tag="scr", name=nm + "negx2")
                negx2 = negx2b[0:1, :].bitcast(F32R)
                nc.scalar.activation(negx2, x2s.bitcast(F32), AF.Copy, scale=-1.0)

                rhsAb = work.tile([66, N], F32R, tag="rhsA", name=nm + "rhsA")
                rhsA = rhsAb[0:rows + 2, :]
                nc.scalar.activation(rhsA[0:rows, :], featr[:].bitcast(F32), AF.Copy)
                nc.sync.dma_start(rhsA[rows:rows + 1, :], x2s)
                nc.sync.dma_start(rhsA[rows + 1:rows + 2, :], onesrow[:, :])
                lhsAb = work.tile([66, N], F32R, tag="lhsA", name=nm + "lhsA")
                lhsA = lhsAb[0:rows + 2, :]
                nc.scalar.activation(lhsA[0:rows, :], featr[:].bitcast(F32),
                                     AF.Copy, scale=2.0)
                nc.sync.dma_start(lhsA[rows:rows + 1, :], negones[:, :])
                nc.sync.dma_start(lhsA[rows + 1:rows + 2, :], negx2)

                idxall = work.tile([128, NT, 8], U32, tag="idxall",
                                   name=nm + "idxall")
                with tc.For_i(0, NT, 1) as t:
                    lstgb = work.tile([66, 128], F32R, tag="lstg", name=nm + "lstg")
                    lstg = lstgb[0:rows + 2, :]
                    nc.scalar.activation(lstg,
                                         lhsA[:, ds(t * 128, 128)].bitcast(F32),
                                         AF.Copy)
                    gps = psp.tile([128, N], F32, tag="ps", name=nm + "gps")
                    for c in range(4):
                        nc.tensor.matmul(gps[:, 512 * c:512 * (c + 1)], lstg,
                                         rhsA[:, 512 * c:512 * (c + 1)])
                    vals = work.tile([128, 8], F32, tag="vals", name=nm + "vals")
                    nc.vector.max(out=vals[:], in_=gps[:])
                    mi = work.tile([128, 8], U32, tag="mi", name=nm + "mi")
                    nc.vector.max_index(out=mi[:], in_max=vals[:], in_values=gps[:])
                    nc.sync.dma_start(idxall[:, ds(t, 1), :], mi[:].unsqueeze(1))

                # redistribute: [128, NT, 8] -> wrapped [16*ngroups, 640]
                idxf = work.tile([128, K, NT], F32, tag="idxf", name=nm + "idxf")
                nc.vector.tensor_copy(idxf[:], idxall[:, :, 0:K].transpose([0, 2, 1]))
                tp = psp.tile([16 * K, 128], F32, tag="ps", name=nm + "tp")
                nc.tensor.transpose(tp[:], idxf[:].rearrange("p a b -> p (a b)"),
                                    ident[:])
                idxt16 = work.tile([16 * K, 128], I16, tag="idxt16",
                                   name=nm + "idxt16")
                nc.vector.tensor_copy(idxt16[:], tp[:])
                wrapb = work.tile([128, 640], I16, tag="wrap", name=nm + "wrap")
                wrap = wrapb[0:16 * ngroups, :]
                for gg in range(ngroups):
                    for k in range(K):
                        nc.sync.dma_start(
                            wrap[16 * gg:16 * (gg + 1), 128 * k:128 * (k + 1)],
                            idxt16[16 * k:16 * k + 16, :])
                return wrapb

            with tc.For_i(0, NG, 1) as g:
                posTr = work.tile([3, N], F32R, tag="posTr", name="posTr")
                nc.scalar.activation(posTr[:], posTall[:, ds(g * N, N)], AF.Copy)

                wrap1 = emit_knn(posTr, 3, ones3, 4, "k1")

                # conv1 node features: B1 natural order, A1 sigma order
                B1Tb = work.tile([128, N], F32, tag="featB", name="B1T")
                B1T = B1Tb[0:64, :]
                psb = psp.tile([64, N], F32, tag="ps", name="psb")
                for c in range(4):
                    nc.tensor.matmul(psb[:, 512 * c:512 * (c + 1)], w1aB,
                                     posTr[:, 512 * c:512 * (c + 1)])
                nc.scalar.activation(B1T, psb[:], AF.Copy)
                A1sb = work.tile([128, N], F32, tag="featA", name="A1s")
                A1s = A1sb[0:64, :]
                psa = psp.tile([64, N], F32, tag="ps", name="psa")
                sig_pos = posTr[:].rearrange("c (b q) -> c q b", b=16, q=128)
                for c in range(4):
                    nc.tensor.matmul(psa[:, 512 * c:512 * (c + 1)], w1aA,
                                     sig_pos[:, 32 * c:32 * (c + 1), :])
                nc.scalar.activation(A1s, psa[:], AF.Copy)

                # conv1 MLP over 5 neighbor slabs, max-aggregate
                maccb = work.tile([128, N], F32, tag="macc", name="macc")
                macc = maccb[0:64, :]
                nc.vector.memset(macc, -1e30)
                with tc.For_i(0, K, 1) as k:
                    idxstg1 = work.tile([128, 128], I16, tag="idxstg",
                                        name="idxstg1")
                    nc.sync.dma_start(idxstg1[0:64, :], wrap1[0:64, ds(k * 128, 128)])
                    g1b = work.tile([128, N], F32, tag="gsl", name="g1")
                    g1 = g1b[0:64, :]
                    nc.gpsimd.ap_gather(
                        out_ap=g1.unsqueeze(-1), in_ap=B1T.unsqueeze(-1),
                        idxs_ap=idxstg1[0:64, :], channels=64, num_elems=N, d=1,
                        num_idxs=N)
                    nc.vector.tensor_tensor(out=g1, in0=g1, in1=A1s, op=ALU.add)
                    r1a = work.tile([64, N], F32R, tag="convr", name="r1a")
                    nc.scalar.activation(r1a[:], g1, AF.Relu, bias=b1a)
                    ps1b = psp.tile([64, N], F32, tag="ps", name="ps1b")
                    for c in range(4):
                        nc.tensor.matmul(ps1b[:, 512 * c:512 * (c + 1)], w1b,
                                         r1a[:, 512 * c:512 * (c + 1)])
                    r1b = work.tile([64, N], F32R, tag="convr", name="r1b")
                    nc.scalar.activation(r1b[:], ps1b[:], AF.Relu, bias=b1b)
                    ps1c = psp.tile([64, N], F32, tag="ps", name="ps1c")
                    for c in range(4):
                        nc.tensor.matmul(ps1c[:, 512 * c:512 * (c + 1)], w1c,
                                         r1b[:, 512 * c:512 * (c + 1)])
                    nc.vector.tensor_tensor(out=macc, in0=macc, in1=ps1c[:],
                                            op=ALU.max)

                # x1 = bn(relu(macc + b1c)) written sigma->natural
                t1b = scr2.tile([64, N], F32, tag="scr", name="t1")
                t1 = t1b[0:64, :]
                nc.scalar.activation(t1, macc, AF.Relu, bias=b1c)
                x1nat = work.tile([64, N], F32R, name="x1nat")
                nc.scalar.activation(
                    x1nat[:].rearrange("c (b q) -> c q b", b=16, q=128),
                    t1.rearrange("c (q b) -> c q b", q=128, b=16),
                    AF.Identity, bias=h1c, scale=s1c)

                wrap2 = emit_knn(x1nat, 64, ones64, 8, "k2")

                # conv2 node features
                B2Tb = work.tile([128, N], F32, tag="featB", name="B2T")
                B2T = B2Tb[:, :]
                psb2 = psp.tile([128, N], F32, tag="ps", name="psb2")
                for c in range(4):
                    nc.tensor.matmul(psb2[:, 512 * c:512 * (c + 1)], w2B,
                                     x1nat[:, 512 * c:512 * (c + 1)])
                nc.scalar.activation(B2T, psb2[:], AF.Copy)
                A2sb = work.tile([128, N], F32, tag="featA", name="A2s")
                A2s = A2sb[:, :]
                psa2 = psp.tile([128, N], F32, tag="ps", name="psa2")
                sig_x1 = x1nat[:].rearrange("c (b q) -> c q b", b=16, q=128)
                for c in range(4):
                    nc.tensor.matmul(psa2[:, 512 * c:512 * (c + 1)], w2A,
                                     sig_x1[:, 32 * c:32 * (c + 1), :])
                nc.scalar.activation(A2s, psa2[:], AF.Copy)

                # conv2 gather-max + combine
                macc2b = work.tile([128, N], F32, tag="macc", name="macc2")
                macc2 = macc2b[:, :]
                nc.vector.memset(macc2, -1e30)
                with tc.For_i(0, K, 1) as k:
                    idxstg2 = work.tile([128, 128], I16, tag="idxstg",
                                        name="idxstg2")
                    nc.sync.dma_start(idxstg2[:], wrap2[:, ds(k * 128, 128)])
                    g2b = work.tile([128, N], F32, tag="gsl", name="g2")
                    g2 = g2b[:, :]
                    nc.gpsimd.ap_gather(
                        out_ap=g2.unsqueeze(-1), in_ap=B2T.unsqueeze(-1),
                        idxs_ap=idxstg2[:], channels=128, num_elems=N, d=1,
                        num_idxs=N)
                    nc.vector.tensor_tensor(out=macc2, in0=macc2, in1=g2,
                                            op=ALU.max)
                nc.vector.tensor_tensor(out=macc2, in0=macc2, in1=A2s, op=ALU.add)
                x2sg = work.tile([128, N], F32R, name="x2sg")
                nc.scalar.activation(x2sg[:], macc2, AF.Relu, bias=b2)

                # linear-l + global max pool (8 tiles of 128 outputs)
                poolg = work.tile([128, 8], F32R, tag="poolg", name="poolg")
                with tc.For_i(0, 8, 1) as mt:
                    wl1stg = work.tile([64, 128], F32R, tag="wl1stg", name="wl1stg")
                    nc.scalar.activation(wl1stg[:],
                                         wl1[:, ds(mt * 128, 128)].bitcast(F32),
                                         AF.Copy)
                    wl2stg = work.tile([128, 128], F32R, tag="wl2stg", name="wl2stg")
                    nc.scalar.activation(wl2stg[:],
                                         wl2[:, ds(mt * 128, 128)].bitcast(F32),
                                         AF.Copy)
                    psl = psp.tile([128, N], F32, tag="ps", name="psl")
                    for c in range(4):
                        nc.tensor.matmul(psl[:, 512 * c:512 * (c + 1)], wl1stg[:],
                                         sig_x1[:, 32 * c:32 * (c + 1), :],
                                         start=True, stop=False)
                    for c in range(4):
                        nc.tensor.matmul(psl[:, 512 * c:512 * (c + 1)], wl2stg[:],
                                         x2sg[:, 512 * c:512 * (c + 1)],
                                         start=False, stop=True)
                    pr = work.tile([128, 1], F32, tag="pr", name="pr")
                    nc.vector.tensor_reduce(pr[:], psl[:], axis=AX.X, op=ALU.max)
                    blstg = work.tile([128, 1], F32, tag="blstg", name="blstg")
                    nc.sync.dma_start(blstg[:], biasw[:, ds(mt + 6, 1)])
                    nc.scalar.activation(poolg[:, ds(mt, 1)], pr[:], AF.Relu,
                                         bias=blstg[:])
                nc.sync.dma_start(poolall[:, :, ds(g, 1)], poolg[:].unsqueeze(-1))

            # ---------------- head MLP (all graphs) ----------------
            rm1 = wp.tile([128, 4, NG], F32R, name="rm1")
            for mt in range(4):
                ph = psp.tile([128, NG], F32, tag="ps", name="ph")
                for kk in range(8):
                    nc.tensor.matmul(ph[:], wm1[:, kk, 128 * mt:128 * (mt + 1)],
                                     poolall[:, kk, :], start=(kk == 0), stop=(kk == 7))
                nc.scalar.activation(rm1[:, mt, :], ph[:], AF.Relu,
                                     bias=biasw[:, 14 + mt:15 + mt])
            rm2 = wp.tile([128, 2, NG], F32R, name="rm2")
            for mt in range(2):
                ph2 = psp.tile([128, NG], F32, tag="ps", name="ph2")
                for kk in range(4):
                    nc.tensor.matmul(ph2[:], wm2[:, kk, 128 * mt:128 * (mt + 1)],
                                     rm1[:, kk, :], start=(kk == 0), stop=(kk == 3))
                nc.scalar.activation(rm2[:, mt, :], ph2[:], AF.Relu,
                                     bias=biasw[:, 18 + mt:19 + mt])
            pho = psp.tile([2, NG], F32, tag="ps", name="pho")
            for kk in range(2):
                nc.tensor.matmul(pho[:], wout[:, kk, :], rm2[:, kk, :],
                                 start=(kk == 0), stop=(kk == 1))
            outs = wp.tile([2, NG], F32, name="outs")
            nc.vector.tensor_scalar_add(outs[:], pho[:], bout)
            nc.sync.dma_start(out_d[:], outs[:])

    nc.compile()
    return nc


def _fold_weights(inp):
    """Host-side BN folding / edge-weight splitting. Layout-only + tiny weight algebra."""
    f = {k: np.asarray(v, dtype=np.float64) for k, v in inp.items()}
    w = {}
    # conv1 layer a: e @ W1a = x_i @ (Wtop - Wbot) + x_j @ Wbot
    w["w1aA"] = (f["w1a"][:3] - f["w1a"][3:])
    w["w1aB"] = f["w1a"][3:]
    w["b1a"] = f["b1a"]
    # fold (s1a, h1a) into layer b; (s1b, h1b) into layer c
    w["w1b"] = f["s1a"][:, None] * f["w1b"]
    w["b1b"] = f["h1a"] @ f["w1b"] + f["b1b"]
    w["w1c"] = f["s1b"][:, None] * f["w1c"]
    w["b1c"] = f["h1b"] @ f["w1c"] + f["b1c"]
    w["s1c"], w["h1c"] = f["s1c"], f["h1c"]
    # conv2
    w["w2A"] = f["w2"][:64] - f["w2"][64:]
    w["w2B"] = f["w2"][64:]
    w["b2"] = f["b2"]
    # linear l: x1-part plain; x2-part folded with (s2, h2)
    w["wl1"] = f["wl"][:64]
    w["wl2"] = f["s2"][:, None] * f["wl"][64:]
    w["bl"] = f["bl"] + f["h2"] @ f["wl"][64:]
    # head: fold (sl, hl) into m1; (sm1, hm1) into m2; (sm2, hm2) into out
    w["wm1"] = f["sl"][:, None] * f["wm1"]
    w["bm1"] = f["hl"] @ f["wm1"] + f["bm1"]
    w["wm2"] = f["sm1"][:, None] * f["wm2"]
    w["bm2"] = f["hm1"] @ f["wm2"] + f["bm2"]
    w["wout"] = f["sm2"][:, None] * f["wout"]
    w["bout"] = f["hm2"] @ f["wout"] + f["bout"]
    return {k: v.astype(np.float32) for k, v in w.items()}


def _pack_wall(w):
    """Pack folded weights into the [128, XT] fp16 wall."""
    wall = np.zeros((128, XT), np.float16)

    def put(col, arr):
        arr = np.asarray(arr, np.float16)
        wall[: arr.shape[0], col:col + arr.shape[1]] = arr

    put(C_W1AA, w["w1aA"])
    put(C_W1AB, w["w1aB"])
    put(C_W1B, w["w1b"])
    put(C_W1C, w["w1c"])
    put(C_W2A, w["w2A"])
    put(C_W2B, w["w2B"])
    put(C_WL1, w["wl1"])
    put(C_WL2, w["wl2"])
    put(C_WM1, np.ascontiguousarray(
        w["wm1"].reshape(8, 128, 512).transpose(1, 0, 2)).reshape(128, 4096))
    put(C_WM2, np.ascontiguousarray(
        w["wm2"].reshape(4, 128, 256).transpose(1, 0, 2)).reshape(128, 1024))
    put(C_WOUT, np.ascontiguousarray(
        w["wout"].reshape(2, 128, 2).transpose(1, 0, 2)).reshape(128, 4))
    put(C_BIAS + 0, w["b1a"].reshape(64, 1))
    put(C_BIAS + 1, w["b1b"].reshape(64, 1))
    put(C_BIAS + 2, w["b1c"].reshape(64, 1))
    put(C_BIAS + 3, w["s1c"].reshape(64, 1))
    put(C_BIAS + 4, w["h1c"].reshape(64, 1))
    put(C_BIAS + 5, w["b2"].reshape(128, 1))
    put(C_BIAS + 6, np.ascontiguousarray(w["bl"].reshape(8, 128).T))
    put(C_BIAS + 14, np.ascontiguousarray(w["bm1"].reshape(4, 128).T))
    put(C_BIAS + 18, np.ascontiguousarray(w["bm2"].reshape(2, 128).T))
    put(C_BIAS + 20, w["bout"].reshape(2, 1))
    return wall


def make_in_maps(inputs):
    w = _fold_weights(inputs)
    wall = _pack_wall(w)
    pos = np.asarray(inputs["pos"], dtype=np.float32)  # [32, 2048, 3]
    in_maps = []
    for c in range(NCORES):
        in_maps.append({
            "posT": np.ascontiguousarray(
                pos[NG * c:NG * (c + 1)].transpose(0, 2, 1)),
            "wsh": np.ascontiguousarray(wall[16 * c:16 * (c + 1)]),
        })
    return in_maps


def kernel(**inputs):
    if "nc" not in _CACHE:
        _CACHE["nc"] = build_nc()
    nc = _CACHE["nc"]

    in_maps = make_in_maps(inputs)
    res = bass_utils.run_bass_kernel_spmd(nc, in_maps, core_ids=list(range(NCORES)))
    B = np.asarray(inputs["pos"]).shape[0]
    out = np.zeros((B, 2), dtype=np.float32)
    for c in range(NCORES):
        out[NG * c:NG * (c + 1)] = res.results[c]["out"].T
    return out
